# revision 41
# baseline (speedup 1.0000x reference)
"""Trainium2 Bass kernel for a 4-layer binary MLP (BinaryFCNN).

Reference computation (per layer):  h = sign_pm1(h @ sign_pm1(W).T + b)
with x: [8192, 4096] fp32, W_l: [4096, 4096] fp32, b_l: [4096] fp32.

Strategy (v2)
-------------
* Data-parallel over the batch: core c processes rows [c*1024, (c+1)*1024).
  No collectives; every core streams all four weight matrices.
* All marshaling that doesn't depend on the matmul results happens on host
  (it is pure re-encoding of the inputs): x is split into fp16 digits
  hi = fp16(x), lo = fp16(x - hi) (the true residual; the PE honors fp16
  subnormals), and each W is pre-encoded as sign fp8 weights
  (w >= 0) - 0.5 in {-0.5, +0.5}.  This removes every DVE/ACT prep op from
  the device loop: the kernel is pure DMA + PE matmul + ACT sign.
* Layer 1 accumulates hi and lo passes into one PSUM tile, sharing the same
  fp8 +-0.5 stationary weights (products are exact exponent shifts, so the
  result carries ~22 significant bits of x; measured a single borderline
  sign flip vs the fp64 oracle).
* Layers 2..4 are bit-exact: +-1 activations and +-0.5 weights in fp8e4m3
  with fp32 PSUM accumulation; fp8 DoubleRow pairs k-chunks for
  2 fp8 MACs/PE/cycle.
* The sign activation uses ACT Sign(2*psum + b) == sign(h @ sign(W).T + b)
  with the per-feature bias as the ACT per-partition bias operand.
* A short burst of throwaway matmuls covers the first weight/x DMAs and
  releases the PE HAM clock gate before the real stream.
* DMA shapes are chosen for >=2 KiB contiguous per-partition lines (finer
  splits measurably slow the whole PE stream) and the first x chunks and
  weight blocks are interleaved at the front of the DMA program order to
  dodge head-of-line blocking on the ~22 GB/s queues.

Per-core floor: 7168 matmuls x 216 ns = 1548 us (layer 1 = 2 fp16-rate
passes, layers 2-4 = 3 fp8 DoubleRow passes); measured 1576 us/core
(vs 1614 us for the previous on-device-prep version).
"""
import numpy as np
import ml_dtypes

import concourse.tile as tile
from concourse import bacc
import concourse.mybir as mybir
from concourse.bass_utils import run_bass_kernel_spmd

F32 = mybir.dt.float32
F16 = mybir.dt.float16
BF16 = mybir.dt.bfloat16
FP8 = mybir.dt.float8e4
SIGN = mybir.ActivationFunctionType.Sign

N_CORES = 8
D_FULL = 4096
B_FULL = 8192
MF = 512  # matmul moving free dim == one fp32 PSUM bank

# HW-validated: fp8e4 +-0.5 stationary with fp16 moving operand is
# bit-identical to fp16 x fp16 at the same speed, and the PE honors fp16
# subnormal moving values exactly (lo = raw residual, no flush-to-zero).
MIXED_W8 = True


def build_binary_mlp(D: int, M: int, n_layers: int = 4) -> "bacc.Bacc":
    """Emit the per-core kernel. D = feature dim, M = per-core batch rows."""
    KO = D // 128  # contraction chunks (also input-feature blocks)
    NB = D // 128  # output-feature blocks
    MH = M // MF   # batch slices of the moving operand

    nc = bacc.Bacc("TRN2", target_bir_lowering=False, debug=False)
    l1_wdt = FP8 if MIXED_W8 else F16
    xhilo = nc.declare_dram_parameter("xhilo", [128, MH, KO, 2, MF], F16, isOutput=False)
    ws = [
        nc.declare_dram_parameter(
            f"w{l + 1}", [NB, 128, KO, 128], FP8 if l > 0 else l1_wdt, isOutput=False
        )
        for l in range(n_layers)
    ]
    bs = [
        nc.declare_dram_parameter(f"b{l + 1}", [128, NB], F32, isOutput=False)
        for l in range(n_layers)
    ]
    out = nc.declare_dram_parameter("out", [NB, 128, M], BF16, isOutput=True)

    with tile.TileContext(nc) as tc:
        with (
            tc.tile_pool(name="const", bufs=1) as const,
            tc.tile_pool(name="wpool", bufs=4) as wpool,
            tc.tile_pool(name="xio", bufs=4) as xio,
            tc.tile_pool(name="psum", bufs=5, space="PSUM") as psum,
            tc.tile_pool(name="psum1", bufs=1, space="PSUM") as psum1,
        ):
            # DMA program order is queue assignment order; the first x chunks
            # (split hi/lo for latency) go absolutely first, then the first
            # weight blocks (split into quarters), then bias (not consumed
            # until ~30us in), so the leading edge of layer 1 starves least.
            hilo = const.tile([128, MH, KO, 2, MF], F16, tag="hilo", name="hilo")
            N_PRE_W = 4
            pre_w = [
                wpool.tile([128, KO, 128], l1_wdt, tag="w", name="wt")
                for _ in range(N_PRE_W)
            ]
            q4 = KO // 4

            def dma_w_quarters(wt, nb):
                for q in range(4):
                    nc.sync.dma_start(wt[:, q * q4:(q + 1) * q4, :],
                                      ws[0][nb, :, q * q4:(q + 1) * q4, :])

            for ko in range(4):  # first chunks split hi/lo for low latency
                nc.sync.dma_start(hilo[:, 0, ko, 0:1, :], xhilo[:, 0, ko, 0:1, :])
                nc.sync.dma_start(hilo[:, 0, ko, 1:2, :], xhilo[:, 0, ko, 1:2, :])
            dma_w_quarters(pre_w[0], 0)
            dma_w_quarters(pre_w[1], 1)
            bias_tiles = []
            for l in range(n_layers):
                bt = const.tile([128, NB], F32, tag=f"bias{l}", name=f"bias{l}")
                nc.sync.dma_start(bt[:], bs[l][:])
                bias_tiles.append(bt)
            for ko in range(4, 8):
                nc.sync.dma_start(hilo[:, 0, ko, :, :], xhilo[:, 0, ko, :, :])
            dma_w_quarters(pre_w[2], 2)
            dma_w_quarters(pre_w[3], 3)
            # bulk x chunks in ko-PAIR pieces: 4 KiB contiguous per-partition
            # lines (vs 2 KiB) for better supply bandwidth where latency no
            # longer matters
            for ko in range(8, KO, 2):
                nc.sync.dma_start(hilo[:, 0, ko:ko + 2, :, :],
                                  xhilo[:, 0, ko:ko + 2, :, :])
            for ko in range(0, KO, 2):
                nc.sync.dma_start(hilo[:, 1, ko:ko + 2, :, :],
                                  xhilo[:, 1, ko:ko + 2, :, :])

            # PE warm-up: covers the first weight/x DMAs and releases the HAM
            # clock gate (cold 1.2 GHz -> warm 2.4 GHz after ~3.4us).
            wu = const.tile([128, MF], F16, tag="warm", name="warm")
            nc.vector.memset(wu[:], 1.0)
            wps = psum1.tile([128, MF], F32, tag="wps", name="wps")
            n_wu = 40
            for i in range(n_wu):
                nc.tensor.matmul(wps[:], wu[:, :128], wu[:],
                                 start=(i == 0), stop=(i == n_wu - 1))

            # layer-1 output activations, feature-major, +-1 in fp8
            hA = const.tile([128, KO, M], FP8, tag="hA", name="hA")

            # ---------------- layer 1: fp16 hi/lo digit passes ----------------
            # mh-outer so the first matmuls only need the first mh slice of x
            # (the second slice streams in behind ~440us of compute); W1 is
            # streamed twice, which the weight DMA hides easily.
            kh = KO // 2
            for mh in range(MH):
                ms = slice(mh * MF, (mh + 1) * MF)
                for nb in range(NB):
                    if mh == 0 and nb < N_PRE_W:
                        wt = pre_w[nb]
                    else:
                        wt = wpool.tile([128, KO, 128], l1_wdt, tag="w", name="wt")
                        nc.sync.dma_start(wt[:, :kh, :], ws[0][nb, :, :kh, :])
                        nc.sync.dma_start(wt[:, kh:, :], ws[0][nb, :, kh:, :])
                    ps = psum.tile([128, MF], F32, tag="ps", name="ps")
                    for ko in range(KO):
                        nc.tensor.matmul(ps[:], wt[:, ko, :], hilo[:, mh, ko, 0, :],
                                         start=(ko == 0), stop=False)
                        nc.tensor.matmul(ps[:], wt[:, ko, :], hilo[:, mh, ko, 1, :],
                                         start=False, stop=(ko == KO - 1))
                    # h1 = Sign(2*psum + b) in {-1, +1} (psum = 0.5 * x@sign(W).T)
                    nc.scalar.activation(hA[:, nb, ms], ps[:], SIGN,
                                         bias=bias_tiles[0][:, nb:nb + 1], scale=2.0)

            # ---------------- layers 2..n: exact +-1 x +-0.5 ----------------
            # second ping-pong activation buffer recycles the x digit buffer
            # (dead once layer 1 is done) to stay within SBUF
            hB = const.tile([128, KO, M], FP8, tag="hilo", name="hB")
            hin, hout = hA, hB
            for l in range(1, n_layers):
                last = l == n_layers - 1
                for nb in range(NB):
                    wt = wpool.tile([128, KO, 128], FP8, tag="w", name="wt")
                    # halves keep 2 KiB per-partition DMA lines; finer splits
                    # drop to 1 KiB lines and starve the PE (measured +300us)
                    nc.sync.dma_start(wt[:, :kh, :], ws[l][nb, :, :kh, :])
                    nc.sync.dma_start(wt[:, kh:, :], ws[l][nb, :, kh:, :])
                    ot = None
                    if last:
                        ot = xio.tile([128, M], BF16, tag="ot", name="ot")
                    # both mh slices accumulate in parallel PSUM banks so each
                    # stationary (LDWEIGHTS) load serves two matmuls
                    pss = [psum.tile([128, MF], F32, tag="ps", name="ps")
                           for _ in range(MH)]
                    for ko in range(0, KO, 2):
                        for mh in range(MH):
                            ms = slice(mh * MF, (mh + 1) * MF)
                            nc.tensor.matmul(
                                pss[mh][:], wt[:, ko:ko + 2, :], hin[:, ko:ko + 2, ms],
                                start=(ko == 0), stop=(ko + 2 == KO),
                                perf_mode=mybir.MatmulPerfMode.DoubleRow)
                    for mh in range(MH):
                        ms = slice(mh * MF, (mh + 1) * MF)
                        if last:
                            nc.scalar.activation(ot[:, ms], pss[mh][:], SIGN,
                                                 bias=bias_tiles[l][:, nb:nb + 1], scale=2.0)
                            # drain in [128, 256] pieces across queues so the
                            # final transfer exposes only ~64 KiB of tail
                            for q in range(2):
                                qs = slice(mh * MF + q * (MF // 2),
                                           mh * MF + (q + 1) * (MF // 2))
                                nc.sync.dma_start(out[nb, :, qs], ot[:, qs])
                        else:
                            nc.scalar.activation(hout[:, nb, ms], pss[mh][:], SIGN,
                                                 bias=bias_tiles[l][:, nb:nb + 1], scale=2.0)
                hin, hout = hout, hin
    nc.compile()
    return nc


def _pack_w(W: np.ndarray, np_dt, paired: bool = False) -> np.ndarray:
    """W [D, D] fp32 -> [NB, 128(p=k_in), KO, 128(n)] sign weights in {-0.5, 0.5}
    with WP[nb, p, ko, n] = 0.5*sign_pm1(W[nb*128 + n, ko*128 + p]).
    paired=True groups output blocks in pairs: [NB//2, 128, 2, KO, 128]."""
    D = W.shape[0]
    nb = D // 128
    sw = np.where(W >= 0, 0.5, -0.5).astype(np_dt)
    arr = sw.reshape(nb, 128, nb, 128).transpose(0, 3, 2, 1)  # [nb, p, ko, n]
    if paired:
        arr = arr.reshape(nb // 2, 2, 128, nb, 128).transpose(0, 2, 1, 3, 4)
    return np.ascontiguousarray(arr)


def _pack_b(b: np.ndarray) -> np.ndarray:
    return np.ascontiguousarray(b.astype(np.float32).reshape(-1, 128).T)


def _pack_hilo(hi_c: np.ndarray, lo_c: np.ndarray, KO: int, MF: int) -> np.ndarray:
    """hi_c, lo_c [D, M] fp16 -> [128, MH, KO, 2, MF] with
    out[p, mh, ko, d, m] = digit_d[ko*128 + p, mh*MF + m]."""
    D, M = hi_c.shape
    MH = M // MF
    a = np.stack([hi_c, lo_c], axis=0)        # [2, D, M]
    a = a.reshape(2, KO, 128, MH, MF)         # [d, ko, p, mh, mf]
    return np.ascontiguousarray(a.transpose(2, 3, 1, 0, 4))


last_result = None  # BassKernelResults of the most recent run (for test.py)
_nc_cache = {}


def kernel(x, W1, b1, W2, b2, W3, b3, W4, b4):
    global last_result
    assert x.shape == (B_FULL, D_FULL)
    M = B_FULL // N_CORES
    KO = D_FULL // 128

    if (D_FULL, M) not in _nc_cache:
        _nc_cache[(D_FULL, M)] = build_binary_mlp(D_FULL, M)
    nc = _nc_cache[(D_FULL, M)]

    # host marshaling: pure re-encoding of the inputs
    xt = np.asarray(x, np.float32).T          # [D, B]
    xt_hi = xt.astype(np.float16)             # 11-bit digit
    xt_lo = (xt - xt_hi.astype(np.float32)).astype(np.float16)  # residual digit

    shared = {}
    l1_np_dt = ml_dtypes.float8_e4m3 if MIXED_W8 else np.float16
    for l, (W, b) in enumerate(((W1, b1), (W2, b2), (W3, b3), (W4, b4)), start=1):
        np_dt = ml_dtypes.float8_e4m3 if l > 1 else l1_np_dt
        shared[f"w{l}"] = _pack_w(np.asarray(W), np_dt)
        shared[f"b{l}"] = _pack_b(np.asarray(b))

    in_maps = []
    for c in range(N_CORES):
        m = dict(shared)
        m["xhilo"] = _pack_hilo(xt_hi[:, c * M:(c + 1) * M],
                                xt_lo[:, c * M:(c + 1) * M], KO, MF)
        in_maps.append(m)

    # retries for transient device hiccups (NRT_EXEC_UNIT_UNRECOVERABLE has
    # been observed sporadically on otherwise healthy workers)
    import time as _time
    res = None
    for attempt in range(3):
        try:
            res = run_bass_kernel_spmd(nc, in_maps, core_ids=list(range(N_CORES)))
            break
        except Exception:
            if attempt == 2:
                raise
            _time.sleep(5.0)
    last_result = res

    parts = []
    for c in range(N_CORES):
        o = np.asarray(res.results[c]["out"])  # [NB, 128, M] bf16, values +-1
        parts.append(o.reshape(D_FULL, M).T)   # -> [M, D] (rows are batch)
    return np.concatenate(parts, axis=0).astype(np.float32)


# revision 43
# speedup vs baseline: 1.1964x; 1.1964x over previous
"""Trainium2 Bass kernel for a 4-layer binary MLP (BinaryFCNN).

Reference computation (per layer):  h = sign_pm1(h @ sign_pm1(W).T + b)
with x: [8192, 4096] fp32, W_l: [4096, 4096] fp32, b_l: [4096] fp32.

Strategy (v2)
-------------
* Data-parallel over the batch: core c processes rows [c*1024, (c+1)*1024).
  No collectives; every core streams all four weight matrices.
* All marshaling that doesn't depend on the matmul results happens on host
  (it is pure re-encoding of the inputs): x is split into fp16 digits
  hi = fp16(x), lo = fp16(x - hi) (the true residual; the PE honors fp16
  subnormals), and each W is pre-encoded as sign fp8 weights
  (w >= 0) - 0.5 in {-0.5, +0.5}.  This removes every DVE/ACT prep op from
  the device loop: the kernel is pure DMA + PE matmul + ACT sign.
* Layer 1 accumulates hi and lo passes into one PSUM tile, sharing the same
  fp8 +-0.5 stationary weights (products are exact exponent shifts, so the
  result carries ~22 significant bits of x; measured a single borderline
  sign flip vs the fp64 oracle).
* Layers 2..4 are bit-exact: +-1 activations and +-0.5 weights in fp8e4m3
  with fp32 PSUM accumulation; fp8 DoubleRow pairs k-chunks for
  2 fp8 MACs/PE/cycle.
* The sign activation uses ACT Sign(2*psum + b) == sign(h @ sign(W).T + b)
  with the per-feature bias as the ACT per-partition bias operand.
* A short burst of throwaway matmuls covers the first weight/x DMAs and
  releases the PE HAM clock gate before the real stream.
* DMA shapes are chosen for >=2 KiB contiguous per-partition lines (finer
  splits measurably slow the whole PE stream) and the first x chunks and
  weight blocks are interleaved at the front of the DMA program order to
  dodge head-of-line blocking on the ~22 GB/s queues.

Per-core floor: 7168 matmuls x 216 ns = 1548 us (layer 1 = 2 fp16-rate
passes, layers 2-4 = 3 fp8 DoubleRow passes); measured 1576 us/core
(vs 1614 us for the previous on-device-prep version).
"""
import numpy as np
import ml_dtypes

import concourse.tile as tile
from concourse import bacc
import concourse.mybir as mybir
from concourse.bass_utils import run_bass_kernel_spmd

F32 = mybir.dt.float32
F16 = mybir.dt.float16
BF16 = mybir.dt.bfloat16
FP8 = mybir.dt.float8e4
SIGN = mybir.ActivationFunctionType.Sign

N_CORES = 8
D_FULL = 4096
B_FULL = 8192
MF = 512  # matmul moving free dim == one fp32 PSUM bank

# HW-validated: fp8e4 +-0.5 stationary with fp16 moving operand is
# bit-identical to fp16 x fp16 at the same speed, and the PE honors fp16
# subnormal moving values exactly (lo = raw residual, no flush-to-zero).
MIXED_W8 = True


def build_binary_mlp(D: int, M: int, n_layers: int = 4) -> "bacc.Bacc":
    """Emit the per-core kernel. D = feature dim, M = per-core batch rows."""
    KO = D // 128  # contraction chunks (also input-feature blocks)
    NB = D // 128  # output-feature blocks
    MH = M // MF   # batch slices of the moving operand

    nc = bacc.Bacc("TRN2", target_bir_lowering=False, debug=False)
    l1_wdt = FP8 if MIXED_W8 else F16
    xhilo = nc.declare_dram_parameter("xhilo", [128, MH, KO, 2, MF], F16, isOutput=False)
    ws = [
        nc.declare_dram_parameter(
            f"w{l + 1}", [NB, 128, KO, 128], FP8 if l > 0 else l1_wdt, isOutput=False
        )
        for l in range(n_layers)
    ]
    bs = [
        nc.declare_dram_parameter(f"b{l + 1}", [128, NB], F32, isOutput=False)
        for l in range(n_layers)
    ]
    out = nc.declare_dram_parameter("out", [NB, 128, M], BF16, isOutput=True)

    with tile.TileContext(nc) as tc:
        with (
            tc.tile_pool(name="const", bufs=1) as const,
            tc.tile_pool(name="wpool", bufs=4) as wpool,
            tc.tile_pool(name="xio", bufs=4) as xio,
            tc.tile_pool(name="psum", bufs=5, space="PSUM") as psum,
            tc.tile_pool(name="psum1", bufs=1, space="PSUM") as psum1,
        ):
            # DMA program order is queue assignment order; the first x chunks
            # (split hi/lo for latency) go absolutely first, then the first
            # weight blocks (split into quarters), then bias (not consumed
            # until ~30us in), so the leading edge of layer 1 starves least.
            hilo = const.tile([128, MH, KO, 2, MF], F16, tag="hilo", name="hilo")
            N_PRE_W = 4
            pre_w = [
                wpool.tile([128, KO, 128], l1_wdt, tag="w", name="wt")
                for _ in range(N_PRE_W)
            ]
            q4 = KO // 4

            def dma_w_quarters(wt, nb):
                for q in range(4):
                    nc.sync.dma_start(wt[:, q * q4:(q + 1) * q4, :],
                                      ws[0][nb, :, q * q4:(q + 1) * q4, :])

            for ko in range(4):  # first chunks split hi/lo for low latency
                nc.sync.dma_start(hilo[:, 0, ko, 0:1, :], xhilo[:, 0, ko, 0:1, :])
                nc.sync.dma_start(hilo[:, 0, ko, 1:2, :], xhilo[:, 0, ko, 1:2, :])
            dma_w_quarters(pre_w[0], 0)
            dma_w_quarters(pre_w[1], 1)
            bias_tiles = []
            for l in range(n_layers):
                bt = const.tile([128, NB], F32, tag=f"bias{l}", name=f"bias{l}")
                nc.sync.dma_start(bt[:], bs[l][:])
                bias_tiles.append(bt)
            for ko in range(4, 8):
                nc.sync.dma_start(hilo[:, 0, ko, :, :], xhilo[:, 0, ko, :, :])
            dma_w_quarters(pre_w[2], 2)
            dma_w_quarters(pre_w[3], 3)
            # bulk x chunks in ko-PAIR pieces: 4 KiB contiguous per-partition
            # lines (vs 2 KiB) for better supply bandwidth where latency no
            # longer matters
            for ko in range(8, KO, 2):
                nc.sync.dma_start(hilo[:, 0, ko:ko + 2, :, :],
                                  xhilo[:, 0, ko:ko + 2, :, :])
            for ko in range(0, KO, 2):
                nc.sync.dma_start(hilo[:, 1, ko:ko + 2, :, :],
                                  xhilo[:, 1, ko:ko + 2, :, :])

            # PE warm-up: covers the first weight/x DMAs and releases the HAM
            # clock gate (cold 1.2 GHz -> warm 2.4 GHz after ~3.4us).
            wu = const.tile([128, MF], F16, tag="warm", name="warm")
            nc.vector.memset(wu[:], 1.0)
            wps = psum1.tile([128, MF], F32, tag="wps", name="wps")
            n_wu = 40
            for i in range(n_wu):
                nc.tensor.matmul(wps[:], wu[:, :128], wu[:],
                                 start=(i == 0), stop=(i == n_wu - 1))

            # layer-1 output activations, feature-major, +-1 in fp8
            hA = const.tile([128, KO, M], FP8, tag="hA", name="hA")

            # ---------------- layer 1: fp16 hi/lo digit passes ----------------
            # mh-outer so the first matmuls only need the first mh slice of x
            # (the second slice streams in behind ~440us of compute); W1 is
            # streamed twice, which the weight DMA hides easily.
            kh = KO // 2
            for mh in range(MH):
                ms = slice(mh * MF, (mh + 1) * MF)
                for nb in range(NB):
                    if mh == 0 and nb < N_PRE_W:
                        wt = pre_w[nb]
                    else:
                        wt = wpool.tile([128, KO, 128], l1_wdt, tag="w", name="wt")
                        nc.sync.dma_start(wt[:, :kh, :], ws[0][nb, :, :kh, :])
                        nc.sync.dma_start(wt[:, kh:, :], ws[0][nb, :, kh:, :])
                    ps = psum.tile([128, MF], F32, tag="ps", name="ps")
                    for ko in range(KO):
                        nc.tensor.matmul(ps[:], wt[:, ko, :], hilo[:, mh, ko, 0, :],
                                         start=(ko == 0), stop=False)
                        nc.tensor.matmul(ps[:], wt[:, ko, :], hilo[:, mh, ko, 1, :],
                                         start=False, stop=(ko == KO - 1))
                    # h1 = Sign(2*psum + b) in {-1, +1} (psum = 0.5 * x@sign(W).T)
                    nc.scalar.activation(hA[:, nb, ms], ps[:], SIGN,
                                         bias=bias_tiles[0][:, nb:nb + 1], scale=2.0)

            # ---------------- layers 2..n: exact +-1 x +-0.5 ----------------
            # second ping-pong activation buffer recycles the x digit buffer
            # (dead once layer 1 is done) to stay within SBUF
            hB = const.tile([128, KO, M], FP8, tag="hilo", name="hB")
            hin, hout = hA, hB
            for l in range(1, n_layers):
                last = l == n_layers - 1
                for nb in range(NB):
                    wt = wpool.tile([128, KO, 128], FP8, tag="w", name="wt")
                    # halves keep 2 KiB per-partition DMA lines; finer splits
                    # drop to 1 KiB lines and starve the PE (measured +300us)
                    nc.sync.dma_start(wt[:, :kh, :], ws[l][nb, :, :kh, :])
                    nc.sync.dma_start(wt[:, kh:, :], ws[l][nb, :, kh:, :])
                    ot = None
                    if last:
                        ot = xio.tile([128, M], BF16, tag="ot", name="ot")
                    # both mh slices accumulate in parallel PSUM banks so each
                    # stationary (LDWEIGHTS) load serves two matmuls
                    pss = [psum.tile([128, MF], F32, tag="ps", name="ps")
                           for _ in range(MH)]
                    for ko in range(0, KO, 2):
                        for mh in range(MH):
                            ms = slice(mh * MF, (mh + 1) * MF)
                            nc.tensor.matmul(
                                pss[mh][:], wt[:, ko:ko + 2, :], hin[:, ko:ko + 2, ms],
                                start=(ko == 0), stop=(ko + 2 == KO),
                                perf_mode=mybir.MatmulPerfMode.DoubleRow)
                    for mh in range(MH):
                        ms = slice(mh * MF, (mh + 1) * MF)
                        if last:
                            nc.scalar.activation(ot[:, ms], pss[mh][:], SIGN,
                                                 bias=bias_tiles[l][:, nb:nb + 1], scale=2.0)
                            # drain in [128, 256] pieces across queues so the
                            # final transfer exposes only ~64 KiB of tail
                            for q in range(2):
                                qs = slice(mh * MF + q * (MF // 2),
                                           mh * MF + (q + 1) * (MF // 2))
                                nc.sync.dma_start(out[nb, :, qs], ot[:, qs])
                        else:
                            nc.scalar.activation(hout[:, nb, ms], pss[mh][:], SIGN,
                                                 bias=bias_tiles[l][:, nb:nb + 1], scale=2.0)
                hin, hout = hout, hin
    nc.compile()
    return nc


def _pack_w(W: np.ndarray, np_dt, paired: bool = False) -> np.ndarray:
    """W [D, D] fp32 -> [NB, 128(p=k_in), KO, 128(n)] sign weights in {-0.5, 0.5}
    with WP[nb, p, ko, n] = 0.5*sign_pm1(W[nb*128 + n, ko*128 + p]).
    paired=True groups output blocks in pairs: [NB//2, 128, 2, KO, 128]."""
    D = W.shape[0]
    nb = D // 128
    sw = np.where(W >= 0, 0.5, -0.5).astype(np_dt)
    arr = sw.reshape(nb, 128, nb, 128).transpose(0, 3, 2, 1)  # [nb, p, ko, n]
    if paired:
        arr = arr.reshape(nb // 2, 2, 128, nb, 128).transpose(0, 2, 1, 3, 4)
    return np.ascontiguousarray(arr)


def _pack_b(b: np.ndarray) -> np.ndarray:
    return np.ascontiguousarray(b.astype(np.float32).reshape(-1, 128).T)


def _pack_hilo(hi_c: np.ndarray, lo_c: np.ndarray, KO: int, MF: int) -> np.ndarray:
    """hi_c, lo_c [D, M] fp16 -> [128, MH, KO, 2, MF] with
    out[p, mh, ko, d, m] = digit_d[ko*128 + p, mh*MF + m]."""
    D, M = hi_c.shape
    MH = M // MF
    a = np.stack([hi_c, lo_c], axis=0)        # [2, D, M]
    a = a.reshape(2, KO, 128, MH, MF)         # [d, ko, p, mh, mf]
    return np.ascontiguousarray(a.transpose(2, 3, 1, 0, 4))


last_result = None  # BassKernelResults of the most recent run (for test.py)
_nc_cache = {}


def kernel(x, W1, b1, W2, b2, W3, b3, W4, b4):
    global last_result
    assert x.shape == (B_FULL, D_FULL)
    M = B_FULL // N_CORES
    KO = D_FULL // 128

    if (D_FULL, M) not in _nc_cache:
        _nc_cache[(D_FULL, M)] = build_binary_mlp(D_FULL, M)
    nc = _nc_cache[(D_FULL, M)]

    # host marshaling: pure re-encoding of the inputs
    xt = np.asarray(x, np.float32).T          # [D, B]
    xt_hi = xt.astype(np.float16)             # 11-bit digit
    xt_lo = (xt - xt_hi.astype(np.float32)).astype(np.float16)  # residual digit

    shared = {}
    l1_np_dt = ml_dtypes.float8_e4m3 if MIXED_W8 else np.float16
    for l, (W, b) in enumerate(((W1, b1), (W2, b2), (W3, b3), (W4, b4)), start=1):
        np_dt = ml_dtypes.float8_e4m3 if l > 1 else l1_np_dt
        shared[f"w{l}"] = _pack_w(np.asarray(W), np_dt)
        shared[f"b{l}"] = _pack_b(np.asarray(b))

    in_maps = []
    for c in range(N_CORES):
        m = dict(shared)
        m["xhilo"] = _pack_hilo(xt_hi[:, c * M:(c + 1) * M],
                                xt_lo[:, c * M:(c + 1) * M], KO, MF)
        in_maps.append(m)

    # retries for transient device hiccups (NRT_EXEC_UNIT_UNRECOVERABLE has
    # been observed sporadically on otherwise healthy workers)
    import time as _time
    res = None
    for attempt in range(3):
        try:
            res = run_bass_kernel_spmd(nc, in_maps, core_ids=list(range(N_CORES)))
            break
        except Exception:
            if attempt == 2:
                raise
            _time.sleep(5.0)
    last_result = res

    parts = []
    for c in range(N_CORES):
        o = np.asarray(res.results[c]["out"])  # [NB, 128, M] bf16, values +-1
        parts.append(o.reshape(D_FULL, M).T)   # -> [M, D] (rows are batch)
    return np.concatenate(parts, axis=0).astype(np.float32)


# revision 44
# speedup vs baseline: 1.1983x; 1.0016x over previous
"""Trainium2 Bass kernel for a 4-layer binary MLP (BinaryFCNN).

Reference computation (per layer):  h = sign_pm1(h @ sign_pm1(W).T + b)
with x: [8192, 4096] fp32, W_l: [4096, 4096] fp32, b_l: [4096] fp32.

Strategy (v2)
-------------
* Data-parallel over the batch: core c processes rows [c*1024, (c+1)*1024).
  No collectives; every core streams all four weight matrices.
* All marshaling that doesn't depend on the matmul results happens on host
  (it is pure re-encoding of the inputs): x is split into fp16 digits
  hi = fp16(x), lo = fp16(x - hi) (the true residual; the PE honors fp16
  subnormals), and each W is pre-encoded as sign fp8 weights
  (w >= 0) - 0.5 in {-0.5, +0.5}.  This removes every DVE/ACT prep op from
  the device loop: the kernel is pure DMA + PE matmul + ACT sign.
* Layer 1 accumulates hi and lo passes into one PSUM tile, sharing the same
  fp8 +-0.5 stationary weights (products are exact exponent shifts, so the
  result carries ~22 significant bits of x; measured a single borderline
  sign flip vs the fp64 oracle).
* Layers 2..4 are bit-exact: +-1 activations and +-0.5 weights in fp8e4m3
  with fp32 PSUM accumulation; fp8 DoubleRow pairs k-chunks for
  2 fp8 MACs/PE/cycle.
* The sign activation uses ACT Sign(2*psum + b) == sign(h @ sign(W).T + b)
  with the per-feature bias as the ACT per-partition bias operand.
* A short burst of throwaway matmuls covers the first weight/x DMAs and
  releases the PE HAM clock gate before the real stream.
* DMA shapes are chosen for >=2 KiB contiguous per-partition lines (finer
  splits measurably slow the whole PE stream) and the first x chunks and
  weight blocks are interleaved at the front of the DMA program order to
  dodge head-of-line blocking on the ~22 GB/s queues.

Per-core floor: 7168 matmuls x 216 ns = 1548 us (layer 1 = 2 fp16-rate
passes, layers 2-4 = 3 fp8 DoubleRow passes); measured 1576 us/core
(vs 1614 us for the previous on-device-prep version).
"""
import numpy as np
import ml_dtypes

import concourse.tile as tile
from concourse import bacc
import concourse.mybir as mybir
from concourse.bass_utils import run_bass_kernel_spmd

F32 = mybir.dt.float32
F16 = mybir.dt.float16
BF16 = mybir.dt.bfloat16
FP8 = mybir.dt.float8e4
SIGN = mybir.ActivationFunctionType.Sign

N_CORES = 8
D_FULL = 4096
B_FULL = 8192
MF = 512  # matmul moving free dim == one fp32 PSUM bank

# HW-validated: fp8e4 +-0.5 stationary with fp16 moving operand is
# bit-identical to fp16 x fp16 at the same speed, and the PE honors fp16
# subnormal moving values exactly (lo = raw residual, no flush-to-zero).
MIXED_W8 = True


def build_binary_mlp(D: int, M: int, n_layers: int = 4) -> "bacc.Bacc":
    """Emit the per-core kernel. D = feature dim, M = per-core batch rows."""
    KO = D // 128  # contraction chunks (also input-feature blocks)
    NB = D // 128  # output-feature blocks
    MH = M // MF   # batch slices of the moving operand

    nc = bacc.Bacc("TRN2", target_bir_lowering=False, debug=False)
    l1_wdt = FP8 if MIXED_W8 else F16
    xhilo = nc.declare_dram_parameter("xhilo", [128, MH, KO, 2, MF], F16, isOutput=False)
    ws = [
        nc.declare_dram_parameter(
            f"w{l + 1}", [NB, 128, KO, 128], FP8 if l > 0 else l1_wdt, isOutput=False
        )
        for l in range(n_layers)
    ]
    bs = [
        nc.declare_dram_parameter(f"b{l + 1}", [128, NB], F32, isOutput=False)
        for l in range(n_layers)
    ]
    out = nc.declare_dram_parameter("out", [NB, 128, M], BF16, isOutput=True)

    with tile.TileContext(nc) as tc:
        with (
            tc.tile_pool(name="const", bufs=1) as const,
            tc.tile_pool(name="wpool", bufs=4) as wpool,
            tc.tile_pool(name="xio", bufs=4) as xio,
            tc.tile_pool(name="psum", bufs=5, space="PSUM") as psum,
            tc.tile_pool(name="psum1", bufs=1, space="PSUM") as psum1,
        ):
            # DMA program order is queue assignment order; the first x chunks
            # (split hi/lo for latency) go absolutely first, then the first
            # weight blocks (split into quarters), then bias (not consumed
            # until ~30us in), so the leading edge of layer 1 starves least.
            hilo = const.tile([128, MH, KO, 2, MF], F16, tag="hilo", name="hilo")
            N_PRE_W = 4
            pre_w = [
                wpool.tile([128, KO, 128], l1_wdt, tag="w", name="wt")
                for _ in range(N_PRE_W)
            ]
            q4 = KO // 4

            def dma_w_quarters(wt, nb):
                for q in range(4):
                    nc.sync.dma_start(wt[:, q * q4:(q + 1) * q4, :],
                                      ws[0][nb, :, q * q4:(q + 1) * q4, :])

            for ko in range(4):  # first chunks split hi/lo for low latency
                nc.sync.dma_start(hilo[:, 0, ko, 0:1, :], xhilo[:, 0, ko, 0:1, :])
                nc.sync.dma_start(hilo[:, 0, ko, 1:2, :], xhilo[:, 0, ko, 1:2, :])
            dma_w_quarters(pre_w[0], 0)
            dma_w_quarters(pre_w[1], 1)
            bias_tiles = []
            for l in range(n_layers):
                bt = const.tile([128, NB], F32, tag=f"bias{l}", name=f"bias{l}")
                nc.sync.dma_start(bt[:], bs[l][:])
                bias_tiles.append(bt)
            for ko in range(4, 8):
                nc.sync.dma_start(hilo[:, 0, ko, :, :], xhilo[:, 0, ko, :, :])
            dma_w_quarters(pre_w[2], 2)
            dma_w_quarters(pre_w[3], 3)
            for ko in range(8, KO):
                nc.sync.dma_start(hilo[:, 0, ko, :, :], xhilo[:, 0, ko, :, :])
            for ko in range(KO):
                nc.sync.dma_start(hilo[:, 1, ko, :, :], xhilo[:, 1, ko, :, :])

            # PE warm-up: covers the first weight/x DMAs and releases the HAM
            # clock gate (cold 1.2 GHz -> warm 2.4 GHz after ~3.4us).
            wu = const.tile([128, MF], F16, tag="warm", name="warm")
            nc.vector.memset(wu[:], 1.0)
            wps = psum1.tile([128, MF], F32, tag="wps", name="wps")
            n_wu = 40
            for i in range(n_wu):
                nc.tensor.matmul(wps[:], wu[:, :128], wu[:],
                                 start=(i == 0), stop=(i == n_wu - 1))

            # layer-1 output activations, feature-major, +-1 in fp8
            hA = const.tile([128, KO, M], FP8, tag="hA", name="hA")

            # ---------------- layer 1: fp16 hi/lo digit passes ----------------
            # mh-outer so the first matmuls only need the first mh slice of x
            # (the second slice streams in behind ~440us of compute); W1 is
            # streamed twice, which the weight DMA hides easily.
            kh = KO // 2
            for mh in range(MH):
                ms = slice(mh * MF, (mh + 1) * MF)
                for nb in range(NB):
                    if mh == 0 and nb < N_PRE_W:
                        wt = pre_w[nb]
                    else:
                        wt = wpool.tile([128, KO, 128], l1_wdt, tag="w", name="wt")
                        nc.sync.dma_start(wt[:, :kh, :], ws[0][nb, :, :kh, :])
                        nc.sync.dma_start(wt[:, kh:, :], ws[0][nb, :, kh:, :])
                    ps = psum.tile([128, MF], F32, tag="ps", name="ps")
                    for ko in range(KO):
                        nc.tensor.matmul(ps[:], wt[:, ko, :], hilo[:, mh, ko, 0, :],
                                         start=(ko == 0), stop=False)
                        nc.tensor.matmul(ps[:], wt[:, ko, :], hilo[:, mh, ko, 1, :],
                                         start=False, stop=(ko == KO - 1))
                    # h1 = Sign(2*psum + b) in {-1, +1} (psum = 0.5 * x@sign(W).T)
                    nc.scalar.activation(hA[:, nb, ms], ps[:], SIGN,
                                         bias=bias_tiles[0][:, nb:nb + 1], scale=2.0)

            # ---------------- layers 2..n: exact +-1 x +-0.5 ----------------
            # second ping-pong activation buffer recycles the x digit buffer
            # (dead once layer 1 is done) to stay within SBUF
            hB = const.tile([128, KO, M], FP8, tag="hilo", name="hB")
            hin, hout = hA, hB
            for l in range(1, n_layers):
                last = l == n_layers - 1
                for nb in range(NB):
                    wt = wpool.tile([128, KO, 128], FP8, tag="w", name="wt")
                    # halves keep 2 KiB per-partition DMA lines; finer splits
                    # drop to 1 KiB lines and starve the PE (measured +300us)
                    nc.sync.dma_start(wt[:, :kh, :], ws[l][nb, :, :kh, :])
                    nc.sync.dma_start(wt[:, kh:, :], ws[l][nb, :, kh:, :])
                    ot = None
                    if last:
                        ot = xio.tile([128, M], BF16, tag="ot", name="ot")
                    # both mh slices accumulate in parallel PSUM banks so each
                    # stationary (LDWEIGHTS) load serves two matmuls
                    pss = [psum.tile([128, MF], F32, tag="ps", name="ps")
                           for _ in range(MH)]
                    for ko in range(0, KO, 2):
                        for mh in range(MH):
                            ms = slice(mh * MF, (mh + 1) * MF)
                            nc.tensor.matmul(
                                pss[mh][:], wt[:, ko:ko + 2, :], hin[:, ko:ko + 2, ms],
                                start=(ko == 0), stop=(ko + 2 == KO),
                                perf_mode=mybir.MatmulPerfMode.DoubleRow)
                    for mh in range(MH):
                        ms = slice(mh * MF, (mh + 1) * MF)
                        if last:
                            nc.scalar.activation(ot[:, ms], pss[mh][:], SIGN,
                                                 bias=bias_tiles[l][:, nb:nb + 1], scale=2.0)
                            # drain in [128, 256] pieces across queues so the
                            # final transfer exposes only ~64 KiB of tail
                            for q in range(2):
                                qs = slice(mh * MF + q * (MF // 2),
                                           mh * MF + (q + 1) * (MF // 2))
                                nc.sync.dma_start(out[nb, :, qs], ot[:, qs])
                        else:
                            nc.scalar.activation(hout[:, nb, ms], pss[mh][:], SIGN,
                                                 bias=bias_tiles[l][:, nb:nb + 1], scale=2.0)
                hin, hout = hout, hin
    nc.compile()
    return nc


def _pack_w(W: np.ndarray, np_dt, paired: bool = False) -> np.ndarray:
    """W [D, D] fp32 -> [NB, 128(p=k_in), KO, 128(n)] sign weights in {-0.5, 0.5}
    with WP[nb, p, ko, n] = 0.5*sign_pm1(W[nb*128 + n, ko*128 + p]).
    paired=True groups output blocks in pairs: [NB//2, 128, 2, KO, 128]."""
    D = W.shape[0]
    nb = D // 128
    sw = np.where(W >= 0, 0.5, -0.5).astype(np_dt)
    arr = sw.reshape(nb, 128, nb, 128).transpose(0, 3, 2, 1)  # [nb, p, ko, n]
    if paired:
        arr = arr.reshape(nb // 2, 2, 128, nb, 128).transpose(0, 2, 1, 3, 4)
    return np.ascontiguousarray(arr)


def _pack_b(b: np.ndarray) -> np.ndarray:
    return np.ascontiguousarray(b.astype(np.float32).reshape(-1, 128).T)


def _pack_hilo(hi_c: np.ndarray, lo_c: np.ndarray, KO: int, MF: int) -> np.ndarray:
    """hi_c, lo_c [D, M] fp16 -> [128, MH, KO, 2, MF] with
    out[p, mh, ko, d, m] = digit_d[ko*128 + p, mh*MF + m]."""
    D, M = hi_c.shape
    MH = M // MF
    a = np.stack([hi_c, lo_c], axis=0)        # [2, D, M]
    a = a.reshape(2, KO, 128, MH, MF)         # [d, ko, p, mh, mf]
    return np.ascontiguousarray(a.transpose(2, 3, 1, 0, 4))


last_result = None  # BassKernelResults of the most recent run (for test.py)
_nc_cache = {}


def kernel(x, W1, b1, W2, b2, W3, b3, W4, b4):
    global last_result
    assert x.shape == (B_FULL, D_FULL)
    M = B_FULL // N_CORES
    KO = D_FULL // 128

    if (D_FULL, M) not in _nc_cache:
        _nc_cache[(D_FULL, M)] = build_binary_mlp(D_FULL, M)
    nc = _nc_cache[(D_FULL, M)]

    # host marshaling: pure re-encoding of the inputs
    xt = np.asarray(x, np.float32).T          # [D, B]
    xt_hi = xt.astype(np.float16)             # 11-bit digit
    xt_lo = (xt - xt_hi.astype(np.float32)).astype(np.float16)  # residual digit

    shared = {}
    l1_np_dt = ml_dtypes.float8_e4m3 if MIXED_W8 else np.float16
    for l, (W, b) in enumerate(((W1, b1), (W2, b2), (W3, b3), (W4, b4)), start=1):
        np_dt = ml_dtypes.float8_e4m3 if l > 1 else l1_np_dt
        shared[f"w{l}"] = _pack_w(np.asarray(W), np_dt)
        shared[f"b{l}"] = _pack_b(np.asarray(b))

    in_maps = []
    for c in range(N_CORES):
        m = dict(shared)
        m["xhilo"] = _pack_hilo(xt_hi[:, c * M:(c + 1) * M],
                                xt_lo[:, c * M:(c + 1) * M], KO, MF)
        in_maps.append(m)

    # retries for transient device hiccups (NRT_EXEC_UNIT_UNRECOVERABLE has
    # been observed sporadically on otherwise healthy workers)
    import time as _time
    res = None
    for attempt in range(3):
        try:
            res = run_bass_kernel_spmd(nc, in_maps, core_ids=list(range(N_CORES)))
            break
        except Exception:
            if attempt == 2:
                raise
            _time.sleep(5.0)
    last_result = res

    parts = []
    for c in range(N_CORES):
        o = np.asarray(res.results[c]["out"])  # [NB, 128, M] bf16, values +-1
        parts.append(o.reshape(D_FULL, M).T)   # -> [M, D] (rows are batch)
    return np.concatenate(parts, axis=0).astype(np.float32)
